# revision 1
# baseline (speedup 1.0000x reference)
"""Bucket (block-diagonal) attention layer for Trainium2, 8 NeuronCores SPMD.

Sharding: data-parallel over batch (4) x tensor-parallel over head groups (2).
Core c = b*2 + g handles batch b, global heads [g*8, g*8+8).

Per-core math (local out dim 512 = 8 heads x 64):
  qT[dl, t] = sum_k Wq[g*512+dl, k] * x[b, t, k]  (+ bq)   [transposed layout]
  kT[dl, t] = likewise (bk dropped: constant-per-row score shifts cancel in
              softmax -- only bq enters scores via bq . k_j)
  v[t, dl]  = natural layout (bf16), with a ones-column appended per head so
              the attended matmul also produces the softmax denominator.
  scoresT[kt, qt] = matmul(lhsT=kT_head, rhs=qT_head)      (K=64)
  expT = exp(scoresT) in bf16 (no max subtraction; logits sigma ~3.3)
  att[qt, 0:64], den[qt] = matmul(lhsT=expT, rhs=[v_head | ones])  (bf16)
  y = att / den + (x_slice + bv)   [residual + bv folded on host, fp16]

Perf structure vs v1 baseline:
 - all attention matmuls 16-bit (v1 ran them fp32 = 4 cycles/row on PE)
 - scores for 4 heads share one PSUM bank -> one batched EXP per [128,512]
 - attended for 4 heads share one bank -> batched reciprocal + strided
   broadcast normalize on DVE (v1: per-head ops)
 - projections of quarter q+1 are emission-interleaved with attention of
   quarter q so the PE stays dense (HAM stays warm) and softmax latency
   hides under projection matmuls.
"""

import json
import sys

import numpy as np

FP16 = np.float16

B, S, D = 4, 4096, 1024
H, NB = 16, 32
HG = 2            # head groups (tensor parallel over heads)
NCORES = B * HG   # 8
DL = D // HG      # 512 local output dims per core
HL = H // HG      # 8 local heads
HD = D // H       # 64 head dim
BS = S // NB      # 128 bucket size
KC = D // 128     # 8 contraction chunks
NQ = 4            # token quarters processed as pipeline phases
TOKQ = S // NQ    # 1024 tokens per quarter
NBQ = TOKQ // BS  # 8 buckets per quarter
OD = DL // 128    # 4 out-dim partition tiles for qT/kT
VW = 66           # per-head block width in v tiles: 64 data + 1 ones + 1 pad

_built = None     # cached (nc,) so repeated kernel() calls reuse the program


def _apply_waitfix():
    """This container's walrus accepts at most ONE sem wait per instruction.
    Post-process the BIR json: hoist extra waits onto injected wait-only
    EventSemaphore instructions just before the owning instruction."""
    import concourse.bass as bass

    if getattr(bass.Bass, "_waitfix_applied", False):
        return
    orig = bass.Bass.to_json_bytes

    def _split(m):
        n = 0
        for f in m["functions"]:
            for blk in f["blocks"]:
                out = []
                for inst in blk["instructions"]:
                    si = inst.get("sync_info")
                    if si and si.get("on_wait") and len(si["on_wait"]) > 1:
                        waits = si["on_wait"]
                        si["on_wait"] = waits[-1:]
                        for k, w in enumerate(waits[:-1]):
                            out.append({
                                "debug": inst.get("debug", 0),
                                "engine": inst["engine"],
                                "ins": [],
                                "outs": [],
                                "name": f"wfix{n}_{k}_{inst['name']}",
                                "opcode": "EventSemaphore",
                                "sync_info": {"on_update": [], "on_wait": [w]},
                            })
                        n += 1
                    out.append(inst)
                blk["instructions"] = out
        return n

    def patched(self):
        m = json.loads(orig(self))
        _split(m)
        return json.dumps(m).encode()

    bass.Bass.to_json_bytes = patched
    bass.Bass._waitfix_applied = True


def _build():
    global _built
    if _built is not None:
        return _built

    _apply_waitfix()
    import concourse.bass as bass
    import concourse.tile as tile
    from concourse import mybir
    from concourse.bass import ts

    f32 = mybir.dt.float32
    fp16 = mybir.dt.float16
    bf16 = mybir.dt.bfloat16
    Act = mybir.ActivationFunctionType
    Alu = mybir.AluOpType

    # All inputs are host-side pre-arranged partition-major so every DMA
    # reads large contiguous spans per partition (small descriptors make
    # the DMA queues descriptor-rate-bound: ~1KB/descr was the v6 head).
    nc = bass.Bass()
    xt = nc.dram_tensor("xt", [128, NQ * KC * TOKQ], fp16,
                        kind="ExternalInput")
    wq = nc.dram_tensor("wq", [128, KC * DL], fp16, kind="ExternalInput")
    wk = nc.dram_tensor("wk", [128, KC * DL], fp16, kind="ExternalInput")
    wv = nc.dram_tensor("wv", [128, KC * DL], fp16, kind="ExternalInput")
    bqt = nc.dram_tensor("bq", [128, OD], f32, kind="ExternalInput")
    xres = nc.dram_tensor("xres", [128, NB * DL], fp16, kind="ExternalInput")
    y = nc.dram_tensor("y", [S, DL], f32, kind="ExternalOutput")

    with tile.TileContext(nc) as tc:
        with (
            tc.tile_pool(name="wpool", bufs=1) as wpool,
            tc.tile_pool(name="xtp", bufs=2) as xtp,
            tc.tile_pool(name="qtp", bufs=2 * OD) as qtp,
            tc.tile_pool(name="ktp", bufs=2 * OD) as ktp,
            tc.tile_pool(name="vp", bufs=2 * NBQ) as vpool,
            tc.tile_pool(name="ep", bufs=4) as epool,
            tc.tile_pool(name="yp", bufs=3) as ypool,
            tc.tile_pool(name="xrp", bufs=2) as xrpool,
            tc.tile_pool(name="rp", bufs=8) as rpool,
            tc.tile_pool(name="ps_p", bufs=2, space="PSUM") as ps_p,
            tc.tile_pool(name="ps_s", bufs=4, space="PSUM") as ps_s,
            tc.tile_pool(name="ps_a", bufs=2, space="PSUM") as ps_a,
        ):
            # --- stationary weights + bias, one big DMA per tensor ---
            # (each dma_start costs ~300ns of descriptor fan-out on the sync
            # sequencer; 25 small weight loads serialized the head)
            # --- PE warm-up: dummy matmuls on zeros during the DMA head so
            # the HAM clock-gate releases (1.2->2.4GHz) before real work ---
            warm = wpool.tile([128, 640], fp16, tag="warm")
            nc.vector.memset(warm[:], 0.0)
            pwarm = ps_s.tile([128, 512], f32, tag="ps", name="pwarm")
            for i in range(8):
                nc.tensor.matmul(pwarm[:], warm[:, 512:640], warm[:, 0:512],
                                 start=(i == 0), stop=(i == 7))
            wsink = wpool.tile([128, 1], f32, tag="wsink")
            nc.vector.reciprocal(wsink[:], pwarm[:, 0:1])

            # Tile dependency tracking is whole-tile granular: anything the
            # first matmuls must not wait for has to live in its OWN tile.
            # So xt quarters and wq are split into kk 0-3 / 4-7 half tiles.
            KH = KC // 2

            def wload(src, nm):
                t = wpool.tile([128, KC, DL], fp16, tag=nm, name=nm)
                ap = src[:, :].rearrange("p (kk d) -> p kk d", kk=KC)
                nc.sync.dma_start(out=t[:], in_=ap)
                return t

            state = {}  # per-quarter tiles: xt, xr, qt, kt, v, ex

            xt4 = xt[:, :].rearrange("p (q kk t) -> p q kk t", q=NQ, kk=KC)
            xr3 = xres[:, :].rearrange("p (nb d) -> p nb d", d=DL)

            def load_xt(q):
                ta = xtp.tile([128, KH, TOKQ], fp16, tag="xta", name="xta")
                tb = xtp.tile([128, KH, TOKQ], fp16, tag="xtb", name="xtb")
                state.setdefault(q, {})["xt"] = (ta, tb)
                nc.sync.dma_start(out=ta[:], in_=xt4[:, q, 0:KH, :])
                nc.sync.dma_start(out=tb[:], in_=xt4[:, q, KH:KC, :])

            def load_xr(q):
                t = xrpool.tile([128, NBQ, DL], fp16, tag="xres", name="xres")
                nc.sync.dma_start(
                    out=t[:], in_=xr3[:, q * NBQ:(q + 1) * NBQ, :])
                state[q]["xr"] = t

            # head order: interleave wq/xt halves so the first unit's
            # matmuls start as soon as wq_a + xt_a (1.5MB) have landed
            wqap = wq[:, :].rearrange("p (kk d) -> p kk d", kk=KC)
            wq_a = wpool.tile([128, KH, DL], fp16, tag="wqa", name="wqa")
            wq_b = wpool.tile([128, KH, DL], fp16, tag="wqb", name="wqb")
            xt0_a = xtp.tile([128, KH, TOKQ], fp16, tag="xta", name="xta")
            xt0_b = xtp.tile([128, KH, TOKQ], fp16, tag="xtb", name="xtb")
            state[0] = {"xt": (xt0_a, xt0_b)}
            nc.sync.dma_start(out=wq_a[:], in_=wqap[:, 0:KH, :])
            nc.sync.dma_start(out=xt0_a[:], in_=xt4[:, 0, 0:KH, :])
            nc.sync.dma_start(out=wq_b[:], in_=wqap[:, KH:KC, :])
            nc.sync.dma_start(out=xt0_b[:], in_=xt4[:, 0, KH:KC, :])
            wk_t = wload(wk, "wk")
            bq_sb = wpool.tile([128, OD], f32, tag="bq")
            nc.sync.dma_start(out=bq_sb[:], in_=bqt[:, :])
            wv_t = wload(wv, "wv")
            load_xr(0)

            def proj_units(q):
                """Yield 24 emission units: 16 q/k groups + 8 v groups."""
                st = state[q]
                xt_ab = st["xt"]
                qt_sb = [qtp.tile([128, TOKQ], fp16, tag="qt", name="qt")
                         for _ in range(OD)]
                kt_sb = [ktp.tile([128, TOKQ], fp16, tag="kt", name="kt")
                         for _ in range(OD)]
                v_sb = [vpool.tile([128, HL * VW], bf16, tag="v", name="v")
                        for _ in range(NBQ)]
                st["qt"], st["kt"], st["v"] = qt_sb, kt_sb, v_sb

                def qk_unit(which, od, tt):
                    def emit():
                        dst = qt_sb if which == "q" else kt_sb
                        p = ps_p.tile([128, 512], f32, tag="pp", name="pp")
                        for kk in range(KC):
                            if which == "q":
                                w_ap = (wq_a if kk < KH else
                                        wq_b)[:, kk % KH, ts(od, 128)]
                            else:
                                w_ap = wk_t[:, kk, ts(od, 128)]
                            nc.tensor.matmul(
                                p[:], w_ap,
                                xt_ab[kk // KH][:, kk % KH, ts(tt, 512)],
                                start=(kk == 0), stop=(kk == KC - 1))
                        if which == "q":
                            nc.scalar.activation(
                                dst[od][:, ts(tt, 512)], p[:], Act.Identity,
                                bias=bq_sb[:, od:od + 1], scale=1.0)
                        else:
                            nc.scalar.copy(dst[od][:, ts(tt, 512)], p[:])
                    return emit

                def v_unit(vt):
                    def emit():
                        p = ps_p.tile([128, 512], f32, tag="pp", name="pp")
                        for kk in range(KC):
                            nc.tensor.matmul(
                                p[:],
                                xt_ab[kk // KH][:, kk % KH, ts(vt, 128)],
                                wv_t[:, kk, :],
                                start=(kk == 0), stop=(kk == KC - 1))
                        vt_sb = v_sb[vt]
                        v3 = vt_sb[:].rearrange("p (h c) -> p h c", c=VW)
                        nc.vector.memset(v3[:, :, 64:66], 1.0)
                        nc.vector.tensor_copy(
                            v3[:, :, 0:64],
                            p[:].rearrange("p (h c) -> p h c", c=HD))
                    return emit

                # tt-major so quarter 0 can start on the first half of xt;
                # q before k so the wk DMA hides under the q-unit stream
                units = []
                for tt in range(2):
                    for od in range(OD):
                        units.append(qk_unit("q", od, tt))
                    for od in range(OD):
                        units.append(qk_unit("k", od, tt))
                for vt in range(NBQ):
                    units.append(v_unit(vt))
                return units

            def attn_scores(q, bk):
                """Part 1: scores matmuls + batched EXP for one bucket."""
                st = state[q]
                qt_sb, kt_sb = st["qt"], st["kt"]
                col = ts(bk, BS)  # token slice within quarter
                se = ps_s.tile([128, 512], f32, tag="ps", name="ps_e")
                so = ps_s.tile([128, 512], f32, tag="ps", name="ps_o")
                # even heads first so the EXP of bank se can start while the
                # odd-head score matmuls still stream
                for h in (0, 2, 4, 6, 1, 3, 5, 7):
                    od, po = h // 2, (h % 2) * 64
                    bank = se if h % 2 == 0 else so
                    nc.tensor.matmul(
                        bank[:, ts(h // 2, 128)],
                        kt_sb[od][po:po + 64, col],
                        qt_sb[od][po:po + 64, col],
                        start=True, stop=True)
                ex_e = epool.tile([128, 512], bf16, tag="ex", name="ex_e")
                ex_o = epool.tile([128, 512], bf16, tag="ex", name="ex_o")
                nc.scalar.activation(ex_e[:], se[:], Act.Exp)
                nc.scalar.activation(ex_o[:], so[:], Act.Exp)
                st.setdefault("ex", {})[bk] = (ex_e, ex_o)

            def attn_out(q, bk):
                """Part 2: attended matmuls + normalize + residual + out."""
                st = state[q]
                v_sb = st["v"]
                ex_e, ex_o = st["ex"].pop(bk)
                tok0 = q * TOKQ
                xr = st["xr"][:, bk, :]
                pe = ps_a.tile([128, HL // 2 * VW], f32, tag="pa", name="pa_e")
                po_ = ps_a.tile([128, HL // 2 * VW], f32, tag="pa", name="pa_o")
                for h in (0, 2, 4, 6, 1, 3, 5, 7):
                    ex = ex_e if h % 2 == 0 else ex_o
                    bank = pe if h % 2 == 0 else po_
                    slot = h // 2
                    nc.tensor.matmul(
                        bank[:, slot * VW:slot * VW + VW],
                        ex[:, ts(slot, 128)],
                        v_sb[bk][:, h * VW:(h + 1) * VW],
                        start=True, stop=True)
                yt = ypool.tile([128, DL], f32, tag="yt")
                for par, bank in ((0, pe), (1, po_)):
                    pav = bank[:].rearrange("p (h c) -> p h c", c=VW)
                    rc = rpool.tile([128, HL // 2], f32, tag="rc")
                    nc.vector.reciprocal(
                        rc[:].unsqueeze(2), pav[:, :, 64:65])
                    ytv = yt[:].rearrange(
                        "p (h two c) -> p h two c", two=2, c=HD)[:, :, par, :]
                    rcb = rc[:].unsqueeze(2).broadcast_to((128, HL // 2, HD))
                    nc.vector.tensor_tensor(
                        out=ytv, in0=pav[:, :, 0:HD], in1=rcb, op=Alu.mult)
                nc.vector.tensor_tensor(
                    out=yt[:], in0=yt[:], in1=xr[:], op=Alu.add)
                nc.sync.dma_start(
                    out=y[tok0 + bk * BS:tok0 + (bk + 1) * BS, :], in_=yt[:])

            # --- emission: per quarter, 16 q/k units then for each bucket
            # [v-unit, scores, attended(bk-1)] -- the EXP latency of bucket
            # bk hides under the v projection of bucket bk+1.  The last
            # bucket's attended spills into the next quarter's first unit.
            pending = None
            for q in range(NQ):
                units = proj_units(q)
                for i in range(2 * OD * 2):
                    units[i]()
                    if i == 0 and pending is not None:
                        attn_out(*pending)
                        pending = None
                    if i == 7 and q + 1 < NQ:
                        load_xt(q + 1)
                        load_xr(q + 1)
                for bk in range(NBQ):
                    units[16 + bk]()
                    attn_scores(q, bk)
                    if bk > 0:
                        attn_out(q, bk - 1)
                pending = (q, NBQ - 1)
            attn_out(*pending)

    _built = nc
    return nc


def _prep_in_maps(x, Wq, bq, Wk, bk, Wv, bv):
    x = np.asarray(x, np.float32)
    Wq = np.asarray(Wq, np.float32)
    Wv = np.asarray(Wv, np.float32)
    Wk = np.asarray(Wk, np.float32)
    bq = np.asarray(bq, np.float32)
    bv = np.asarray(bv, np.float32)

    # partition-major layouts (see dram_tensor comments in _build):
    #   xt  [128, NQ, KC, TOKQ]: [p, q, kk, t] = x.T[kk*128+p, q*TOKQ+t]
    #   w*  [128, KC, DL]:       [p, kk, d]    = W.T[kk*128+p, d]
    #   xres[128, NB, DL]:       [p, nb, d]    = x[nb*128+p, d] + bv[d]
    def _xt_layout(a):  # a: [D, S]
        return np.ascontiguousarray(
            a.reshape(KC, 128, NQ, TOKQ).transpose(1, 2, 0, 3)
        ).reshape(128, NQ * KC * TOKQ)

    def _w_layout(a):  # a: [D, DL]
        return np.ascontiguousarray(
            a.reshape(KC, 128, DL).transpose(1, 0, 2)).reshape(128, KC * DL)

    xt_b = [_xt_layout(x[b].T.astype(FP16)) for b in range(B)]
    wq_g, wk_g, wv_g, bq_g = [], [], [], []
    for g in range(HG):
        sl = slice(g * DL, (g + 1) * DL)
        wq_g.append(_w_layout(Wq[sl, :].T.astype(FP16)))
        wk_g.append(_w_layout(Wk[sl, :].T.astype(FP16)))
        wv_g.append(_w_layout(Wv[sl, :].T.astype(FP16)))
        bq_g.append(np.ascontiguousarray(
            bq[sl].reshape(DL // 128, 128).T).astype(np.float32))

    in_maps = []
    for c in range(NCORES):
        b, g = c // HG, c % HG
        sl = slice(g * DL, (g + 1) * DL)
        xres = (x[b][:, sl] + bv[None, sl]).astype(FP16)  # [S, DL]
        xres = np.ascontiguousarray(
            xres.reshape(NB, 128, DL).transpose(1, 0, 2)
        ).reshape(128, NB * DL)
        in_maps.append({
            "xt": xt_b[b], "wq": wq_g[g], "wk": wk_g[g], "wv": wv_g[g],
            "bq": bq_g[g], "xres": xres,
        })
    return in_maps


def _gather(results):
    out = np.empty((B, S, D), np.float32)
    for c, r in enumerate(results):
        b, g = c // HG, c % HG
        out[b, :, g * DL:(g + 1) * DL] = r["y"]
    return out


def _run(inputs, trace=False, trace_cores=None):
    nc = _build()
    from concourse.bass_utils import run_bass_kernel_spmd

    in_maps = _prep_in_maps(**inputs)
    res = run_bass_kernel_spmd(
        nc, in_maps, core_ids=list(range(NCORES)), trace=trace,
        trace_cores=trace_cores)
    return _gather(res.results), res


def kernel(**inputs):
    out, _ = _run(inputs, trace=False)
    return out


def kernel_traced(trace_cores=None, **inputs):
    """For test.py: returns (output, BassKernelResults with exec_time_ns)."""
    import types
    import trn_agent_boot.trn_boot as tb

    if "antenv.axon_hooks" not in sys.modules:
        hooks = types.ModuleType("antenv.axon_hooks")
        state = [None]
        hooks.set_axon_ntff_profile_hook = lambda h: state.__setitem__(0, h)
        hooks.get_axon_ntff_profile_hook = lambda: state[0]
        sys.modules["antenv.axon_hooks"] = hooks
        hooks.set_axon_ntff_profile_hook(
            tb._ntff_profile_via_ctypes("/opt/axon/libaxon_pjrt.so"))
    return _run(inputs, trace=True, trace_cores=trace_cores)



# revision 2
# speedup vs baseline: 1.0968x; 1.0968x over previous
"""Bucket (block-diagonal) attention layer for Trainium2, 8 NeuronCores SPMD.

Sharding: data-parallel over batch (4) x tensor-parallel over head groups (2).
Core c = b*2 + g handles batch b, global heads [g*8, g*8+8).

Per-core math (local out dim 512 = 8 heads x 64):
  qT[dl, t] = sum_k Wq[g*512+dl, k] * x[b, t, k]  (+ bq)   [transposed layout]
  kT[dl, t] = likewise (bk dropped: constant-per-row score shifts cancel in
              softmax -- only bq enters scores via bq . k_j)
  v[t, dl]  = fp8-e4m3 DoubleRow matmul (x*SX, Wv*SW quantized on host); the
              ones-columns are memset to ALPHA=SX*SW so attended/den descales
              exactly for free.
  scoresT[kt, qt] = matmul(lhsT=kT_head, rhs=qT_head)      (K=64)
  expT = exp(scoresT) in bf16 (no max subtraction; logits sigma ~3.3)
  att[qt, 0:64], den[qt] = matmul(lhsT=expT, rhs=[v_head | alpha])  (bf16)
  y = att / den + (x_slice + bv)   [residual + bv folded on host, fp16]

Perf structure vs v7 baseline (219us):
 - V projection runs fp8 DoubleRow: 4 K=256 matmuls per 128-token tile
   instead of 8 K=128 fp16 matmuls (PE ~1.5x on that third of proj work)
 - score matmuls emitted h=0..7 so consecutive MMs target alternating
   row-groups (K=64 -> rows 0-63 / 64-127) and stream concurrently
 - xt DRAM layout made tt-contiguous; head DMAs reordered/split so the PE
   never waits >1 load at the start
 - y written fp16 (halves writeback bytes; host casts to fp32)
"""

import json
import sys

import numpy as np

FP16 = np.float16

B, S, D = 4, 4096, 1024
H, NB = 16, 32
HG = 2            # head groups (tensor parallel over heads)
NCORES = B * HG   # 8
DL = D // HG      # 512 local output dims per core
HL = H // HG      # 8 local heads
HD = D // H       # 64 head dim
BS = S // NB      # 128 bucket size
KC = D // 128     # 8 contraction chunks of 128
KH = KC // 2      # 4 chunks per a/b half tile
CC = D // 256     # 4 fp8 DoubleRow contraction chunks of 256
NQ = 4            # token quarters processed as pipeline phases
TOKQ = S // NQ    # 1024 tokens per quarter
NBQ = TOKQ // BS  # 8 buckets per quarter
OD = DL // 128    # 4 out-dim partition tiles for qT/kT
VW = 66           # per-head block width in v tiles: 64 data + 1 ones + 1 pad

SX = 28.0         # fp8 scale for x  (|x|max*28 ~ 148 < 240)
SW = 768.0        # fp8 scale for Wv (|Wv|max*768 ~ 81 < 240)
ALPHA = SX * SW   # 21504 = 1.3125*2^14, exact in bf16

_built = None     # cached (nc,) so repeated kernel() calls reuse the program


def _apply_waitfix():
    """This container's walrus accepts at most ONE sem wait per instruction.
    Post-process the BIR json: hoist extra waits onto injected wait-only
    EventSemaphore instructions just before the owning instruction."""
    import concourse.bass as bass

    if getattr(bass.Bass, "_waitfix_applied", False):
        return
    orig = bass.Bass.to_json_bytes

    def _split(m):
        n = 0
        for f in m["functions"]:
            for blk in f["blocks"]:
                out = []
                for inst in blk["instructions"]:
                    si = inst.get("sync_info")
                    if si and si.get("on_wait") and len(si["on_wait"]) > 1:
                        waits = si["on_wait"]
                        si["on_wait"] = waits[-1:]
                        for k, w in enumerate(waits[:-1]):
                            out.append({
                                "debug": inst.get("debug", 0),
                                "engine": inst["engine"],
                                "ins": [],
                                "outs": [],
                                "name": f"wfix{n}_{k}_{inst['name']}",
                                "opcode": "EventSemaphore",
                                "sync_info": {"on_update": [], "on_wait": [w]},
                            })
                        n += 1
                    out.append(inst)
                blk["instructions"] = out
        return n

    def patched(self):
        m = json.loads(orig(self))
        _split(m)
        return json.dumps(m).encode()

    bass.Bass.to_json_bytes = patched
    bass.Bass._waitfix_applied = True


def _build():
    global _built
    if _built is not None:
        return _built

    _apply_waitfix()
    import concourse.bass as bass
    import concourse.tile as tile
    from concourse import mybir
    from concourse.bass import ts

    f32 = mybir.dt.float32
    fp16 = mybir.dt.float16
    bf16 = mybir.dt.bfloat16
    f8e4 = mybir.dt.float8e4
    Act = mybir.ActivationFunctionType
    Alu = mybir.AluOpType
    DR = mybir.MatmulPerfMode.DoubleRow

    # All inputs are host-side pre-arranged partition-major so every DMA
    # reads large contiguous spans per partition.
    #   xt  [p, q, tt, ab, kk, t]: x.T[(ab*KH+kk)*128+p, q*1024+tt*512+t] fp16
    #   xv8 [p, q, cc, i, t]:      e4m3(SX * x.T[cc*256+i*128+p, q*1024+t])
    #   wq/wk [p, kk, d]:          W.T[kk*128+p, d] fp16
    #   wv8 [p, cc, i, d]:         e4m3(SW * Wv.T[cc*256+i*128+p, d])
    #   xres[p, nb, d]:            x[nb*128+p, d] + bv[d] fp16
    nc = bass.Bass()
    xt = nc.dram_tensor("xt", [128, NQ * 2 * 2 * KH * 512], fp16,
                        kind="ExternalInput")
    xv8 = nc.dram_tensor("xv8", [128, NQ * CC * 2 * TOKQ], f8e4,
                         kind="ExternalInput")
    wq = nc.dram_tensor("wq", [128, KC * DL], fp16, kind="ExternalInput")
    wk = nc.dram_tensor("wk", [128, KC * DL], fp16, kind="ExternalInput")
    wv8d = nc.dram_tensor("wv8", [128, CC * 2 * DL], f8e4,
                          kind="ExternalInput")
    bqt = nc.dram_tensor("bq", [128, OD], f32, kind="ExternalInput")
    xres = nc.dram_tensor("xres", [128, NB * DL], fp16, kind="ExternalInput")
    y = nc.dram_tensor("y", [S, DL], fp16, kind="ExternalOutput")

    with tile.TileContext(nc) as tc:
        with (
            tc.tile_pool(name="wpool", bufs=1) as wpool,
            tc.tile_pool(name="xtp", bufs=2) as xtp,
            tc.tile_pool(name="xvp", bufs=2) as xvp,
            tc.tile_pool(name="qtp", bufs=2 * OD) as qtp,
            tc.tile_pool(name="ktp", bufs=2 * OD) as ktp,
            tc.tile_pool(name="vp", bufs=2 * NBQ) as vpool,
            tc.tile_pool(name="ep", bufs=4) as epool,
            tc.tile_pool(name="yp", bufs=3) as ypool,
            tc.tile_pool(name="xrp", bufs=2) as xrpool,
            tc.tile_pool(name="rp", bufs=8) as rpool,
            tc.tile_pool(name="ps_p", bufs=2, space="PSUM") as ps_p,
            tc.tile_pool(name="ps_s", bufs=4, space="PSUM") as ps_s,
            tc.tile_pool(name="ps_a", bufs=2, space="PSUM") as ps_a,
        ):
            # --- PE warm-up: dummy matmuls on zeros during the DMA head so
            # the HAM clock-gate releases (1.2->2.4GHz) before real work ---
            warm = wpool.tile([128, 640], fp16, tag="warm")
            nc.vector.memset(warm[:], 0.0)
            pwarm = ps_s.tile([128, 512], f32, tag="ps", name="pwarm")
            for i in range(8):
                nc.tensor.matmul(pwarm[:], warm[:, 512:640], warm[:, 0:512],
                                 start=(i == 0), stop=(i == 7))
            wsink = wpool.tile([128, 1], f32, tag="wsink")
            nc.vector.reciprocal(wsink[:], pwarm[:, 0:1])

            xt6 = xt[:, :].rearrange(
                "p (q tt ab kk t) -> p q tt ab kk t", q=NQ, tt=2, ab=2, kk=KH)
            xv5 = xv8[:, :].rearrange(
                "p (q cc i t) -> p q cc i t", q=NQ, cc=CC, i=2)
            xr3 = xres[:, :].rearrange("p (nb d) -> p nb d", d=DL)
            wqap = wq[:, :].rearrange("p (kk d) -> p kk d", kk=KC)
            wkap = wk[:, :].rearrange("p (kk d) -> p kk d", kk=KC)
            wv3 = wv8d[:, :].rearrange("p (cc i d) -> p cc i d", cc=CC, i=2)

            state = {}  # per-quarter tiles: xt, xv, xr, qt, kt, v, ex

            def load_xt(q):
                tiles = []
                for tt in range(2):
                    for ab in range(2):
                        t = xtp.tile([128, KH, 512], fp16,
                                     tag=f"xt{tt}{ab}", name=f"xt{tt}{ab}")
                        nc.sync.dma_start(out=t[:], in_=xt6[:, q, tt, ab, :, :])
                        tiles.append(t)
                state.setdefault(q, {})["xt"] = tiles

            def load_xv(q):
                t = xvp.tile([128, CC, 2, TOKQ], f8e4, tag="xv", name="xv")
                nc.sync.dma_start(out=t[:], in_=xv5[:, q, :, :, :])
                state[q]["xv"] = t

            def load_xr(q):
                t = xrpool.tile([128, NBQ, DL], fp16, tag="xres", name="xres")
                nc.sync.dma_start(
                    out=t[:], in_=xr3[:, q * NBQ:(q + 1) * NBQ, :])
                state[q]["xr"] = t

            # --- head: interleave weight/x loads so the first q-units can
            # start after ~1MB and never stall afterwards ---
            bq_sb = wpool.tile([128, OD], f32, tag="bq")
            nc.sync.dma_start(out=bq_sb[:], in_=bqt[:, :])
            wq_a = wpool.tile([128, KH, DL], fp16, tag="wqa", name="wqa")
            wq_b = wpool.tile([128, KH, DL], fp16, tag="wqb", name="wqb")
            wk_a = wpool.tile([128, KH, DL], fp16, tag="wka", name="wka")
            wk_b = wpool.tile([128, KH, DL], fp16, tag="wkb", name="wkb")
            wv8_t = wpool.tile([128, CC, 2, DL], f8e4, tag="wv8", name="wv8")
            xt00a = xtp.tile([128, KH, 512], fp16, tag="xt00", name="xt00")
            xt00b = xtp.tile([128, KH, 512], fp16, tag="xt01", name="xt01")
            xt01a = xtp.tile([128, KH, 512], fp16, tag="xt10", name="xt10")
            xt01b = xtp.tile([128, KH, 512], fp16, tag="xt11", name="xt11")
            state[0] = {"xt": [xt00a, xt00b, xt01a, xt01b]}
            nc.sync.dma_start(out=wq_a[:], in_=wqap[:, 0:KH, :])
            nc.sync.dma_start(out=xt00a[:], in_=xt6[:, 0, 0, 0, :, :])
            nc.sync.dma_start(out=wq_b[:], in_=wqap[:, KH:KC, :])
            nc.sync.dma_start(out=xt00b[:], in_=xt6[:, 0, 0, 1, :, :])
            nc.sync.dma_start(out=wk_a[:], in_=wkap[:, 0:KH, :])
            nc.sync.dma_start(out=wk_b[:], in_=wkap[:, KH:KC, :])
            nc.sync.dma_start(out=xt01a[:], in_=xt6[:, 0, 1, 0, :, :])
            nc.sync.dma_start(out=xt01b[:], in_=xt6[:, 0, 1, 1, :, :])
            nc.sync.dma_start(out=wv8_t[:], in_=wv3[:, :, :, :])
            load_xv(0)
            load_xr(0)

            def proj_units(q):
                """Yield 24 emission units: 16 q/k groups + 8 v groups."""
                st = state[q]
                xts = st["xt"]
                qt_sb = [qtp.tile([128, TOKQ], fp16, tag="qt", name="qt")
                         for _ in range(OD)]
                kt_sb = [ktp.tile([128, TOKQ], fp16, tag="kt", name="kt")
                         for _ in range(OD)]
                v_sb = [vpool.tile([128, HL * VW], bf16, tag="v", name="v")
                        for _ in range(NBQ)]
                st["qt"], st["kt"], st["v"] = qt_sb, kt_sb, v_sb

                def qk_unit(which, od, tt):
                    def emit():
                        dst = qt_sb if which == "q" else kt_sb
                        wa, wb = (wq_a, wq_b) if which == "q" else (wk_a, wk_b)
                        p = ps_p.tile([128, 512], f32, tag="pp", name="pp")
                        for kk in range(KC):
                            w_ap = (wa if kk < KH else wb)[
                                :, kk % KH, ts(od, 128)]
                            nc.tensor.matmul(
                                p[:], w_ap, xts[tt * 2 + kk // KH][:, kk % KH, :],
                                start=(kk == 0), stop=(kk == KC - 1))
                        if which == "q":
                            nc.scalar.activation(
                                dst[od][:, ts(tt, 512)], p[:], Act.Identity,
                                bias=bq_sb[:, od:od + 1], scale=1.0)
                        else:
                            nc.scalar.copy(dst[od][:, ts(tt, 512)], p[:])
                    return emit

                def v_unit(vt):
                    def emit():
                        xv = st["xv"]
                        p = ps_p.tile([128, 512], f32, tag="pp", name="pp")
                        for cc in range(CC):
                            nc.tensor.matmul(
                                p[:], xv[:, cc, :, ts(vt, 128)],
                                wv8_t[:, cc, :, :],
                                start=(cc == 0), stop=(cc == CC - 1),
                                perf_mode=DR)
                        vt_sb = v_sb[vt]
                        v3 = vt_sb[:].rearrange("p (h c) -> p h c", c=VW)
                        nc.vector.memset(v3[:, :, 64:66], ALPHA)
                        nc.vector.tensor_copy(
                            v3[:, :, 0:64],
                            p[:].rearrange("p (h c) -> p h c", c=HD))
                    return emit

                # tt-major so quarter 0 can start on the first xt tile;
                # q before k so the wk DMA hides under the q-unit stream
                units = []
                for tt in range(2):
                    for od in range(OD):
                        units.append(qk_unit("q", od, tt))
                    for od in range(OD):
                        units.append(qk_unit("k", od, tt))
                for vt in range(NBQ):
                    units.append(v_unit(vt))
                return units

            def attn_scores(q, bk):
                """Part 1: scores matmuls + batched EXP for one bucket."""
                st = state[q]
                qt_sb, kt_sb = st["qt"], st["kt"]
                col = ts(bk, BS)  # token slice within quarter
                se = ps_s.tile([128, 512], f32, tag="ps", name="ps_e")
                so = ps_s.tile([128, 512], f32, tag="ps", name="ps_o")
                # h order 0..7: consecutive MMs target alternating row-groups
                # (rows 0-63 / 64-127) so pairs stream concurrently on the PE
                for h in range(HL):
                    od, po = h // 2, (h % 2) * 64
                    bank = se if h % 2 == 0 else so
                    nc.tensor.matmul(
                        bank[:, ts(h // 2, 128)],
                        kt_sb[od][po:po + 64, col],
                        qt_sb[od][po:po + 64, col],
                        start=True, stop=True)
                ex_e = epool.tile([128, 512], bf16, tag="ex", name="ex_e")
                ex_o = epool.tile([128, 512], bf16, tag="ex", name="ex_o")
                nc.scalar.activation(ex_e[:], se[:], Act.Exp)
                nc.scalar.activation(ex_o[:], so[:], Act.Exp)
                st.setdefault("ex", {})[bk] = (ex_e, ex_o)

            def attn_out(q, bk):
                """Part 2: attended matmuls + normalize + residual + out."""
                st = state[q]
                v_sb = st["v"]
                ex_e, ex_o = st["ex"].pop(bk)
                tok0 = q * TOKQ
                xr = st["xr"][:, bk, :]
                pe = ps_a.tile([128, HL // 2 * VW], f32, tag="pa", name="pa_e")
                po_ = ps_a.tile([128, HL // 2 * VW], f32, tag="pa", name="pa_o")
                for h in (0, 2, 4, 6, 1, 3, 5, 7):
                    ex = ex_e if h % 2 == 0 else ex_o
                    bank = pe if h % 2 == 0 else po_
                    slot = h // 2
                    nc.tensor.matmul(
                        bank[:, slot * VW:slot * VW + VW],
                        ex[:, ts(slot, 128)],
                        v_sb[bk][:, h * VW:(h + 1) * VW],
                        start=True, stop=True)
                yt = ypool.tile([128, DL], fp16, tag="yt")
                for par, bank in ((0, pe), (1, po_)):
                    pav = bank[:].rearrange("p (h c) -> p h c", c=VW)
                    rc = rpool.tile([128, HL // 2], f32, tag="rc")
                    nc.vector.reciprocal(
                        rc[:].unsqueeze(2), pav[:, :, 64:65])
                    ytv = yt[:].rearrange(
                        "p (h two c) -> p h two c", two=2, c=HD)[:, :, par, :]
                    rcb = rc[:].unsqueeze(2).broadcast_to((128, HL // 2, HD))
                    nc.vector.tensor_tensor(
                        out=ytv, in0=pav[:, :, 0:HD], in1=rcb, op=Alu.mult)
                nc.vector.tensor_tensor(
                    out=yt[:], in0=yt[:], in1=xr[:], op=Alu.add)
                nc.sync.dma_start(
                    out=y[tok0 + bk * BS:tok0 + (bk + 1) * BS, :], in_=yt[:])

            # --- emission: per quarter, 16 q/k units then for each bucket
            # [v-unit, scores, attended(bk-1)] -- the EXP latency of bucket
            # bk hides under the v projection of bucket bk+1.  The last
            # bucket's attended spills into the next quarter's first unit.
            pending = None
            for q in range(NQ):
                units = proj_units(q)
                for i in range(2 * OD * 2):
                    units[i]()
                    if i == 0 and pending is not None:
                        attn_out(*pending)
                        pending = None
                    if i == 7 and q + 1 < NQ:
                        load_xt(q + 1)
                        load_xv(q + 1)
                        load_xr(q + 1)
                for bk in range(NBQ):
                    units[16 + bk]()
                    attn_scores(q, bk)
                    if bk > 0:
                        attn_out(q, bk - 1)
                pending = (q, NBQ - 1)
            attn_out(*pending)

    _built = nc
    return nc


def _prep_in_maps(x, Wq, bq, Wk, bk, Wv, bv):
    import ml_dtypes

    E4M3 = ml_dtypes.float8_e4m3

    x = np.asarray(x, np.float32)
    Wq = np.asarray(Wq, np.float32)
    Wv = np.asarray(Wv, np.float32)
    Wk = np.asarray(Wk, np.float32)
    bq = np.asarray(bq, np.float32)
    bv = np.asarray(bv, np.float32)

    def _xt_layout(a):  # a: [D, S] fp16 -> (p, q, tt, ab, kk, t)
        return np.ascontiguousarray(
            a.reshape(2, KH, 128, NQ, 2, 512).transpose(2, 3, 4, 0, 1, 5)
        ).reshape(128, NQ * 2 * 2 * KH * 512)

    def _xv8_layout(aT):  # aT: [D, S] fp32 -> (p, q, cc, i, t) e4m3*SX
        a = np.asarray(aT * np.float32(SX), E4M3)
        return np.ascontiguousarray(
            a.reshape(CC, 2, 128, NQ, TOKQ).transpose(2, 3, 0, 1, 4)
        ).reshape(128, NQ * CC * 2 * TOKQ)

    def _w_layout(a):  # a: [D, DL] fp16 -> (p, kk, d)
        return np.ascontiguousarray(
            a.reshape(KC, 128, DL).transpose(1, 0, 2)).reshape(128, KC * DL)

    def _wv8_layout(aT):  # aT: [D, DL] fp32 -> (p, cc, i, d) e4m3*SW
        a = np.asarray(aT * np.float32(SW), E4M3)
        return np.ascontiguousarray(
            a.reshape(CC, 2, 128, DL).transpose(2, 0, 1, 3)
        ).reshape(128, CC * 2 * DL)

    xT_b = [np.ascontiguousarray(x[b].T) for b in range(B)]
    xt_b = [_xt_layout(t.astype(FP16)) for t in xT_b]
    xv8_b = [_xv8_layout(t) for t in xT_b]
    wq_g, wk_g, wv8_g, bq_g = [], [], [], []
    for g in range(HG):
        sl = slice(g * DL, (g + 1) * DL)
        wq_g.append(_w_layout(Wq[sl, :].T.astype(FP16)))
        wk_g.append(_w_layout(Wk[sl, :].T.astype(FP16)))
        wv8_g.append(_wv8_layout(np.ascontiguousarray(Wv[sl, :].T)))
        bq_g.append(np.ascontiguousarray(
            bq[sl].reshape(DL // 128, 128).T).astype(np.float32))

    in_maps = []
    for c in range(NCORES):
        b, g = c // HG, c % HG
        sl = slice(g * DL, (g + 1) * DL)
        xres = (x[b][:, sl] + bv[None, sl]).astype(FP16)  # [S, DL]
        xres = np.ascontiguousarray(
            xres.reshape(NB, 128, DL).transpose(1, 0, 2)
        ).reshape(128, NB * DL)
        in_maps.append({
            "xt": xt_b[b], "xv8": xv8_b[b], "wq": wq_g[g], "wk": wk_g[g],
            "wv8": wv8_g[g], "bq": bq_g[g], "xres": xres,
        })
    return in_maps


def _gather(results):
    out = np.empty((B, S, D), np.float32)
    for c, r in enumerate(results):
        b, g = c // HG, c % HG
        out[b, :, g * DL:(g + 1) * DL] = np.asarray(r["y"], np.float32)
    return out


def _run(inputs, trace=False, trace_cores=None):
    nc = _build()
    from concourse.bass_utils import run_bass_kernel_spmd

    in_maps = _prep_in_maps(**inputs)
    res = run_bass_kernel_spmd(
        nc, in_maps, core_ids=list(range(NCORES)), trace=trace,
        trace_cores=trace_cores)
    return _gather(res.results), res


def kernel(**inputs):
    out, _ = _run(inputs, trace=False)
    return out


def kernel_traced(trace_cores=None, **inputs):
    """For test.py: returns (output, BassKernelResults with exec_time_ns)."""
    import types
    import trn_agent_boot.trn_boot as tb

    if "antenv.axon_hooks" not in sys.modules:
        hooks = types.ModuleType("antenv.axon_hooks")
        state = [None]
        hooks.set_axon_ntff_profile_hook = lambda h: state.__setitem__(0, h)
        hooks.get_axon_ntff_profile_hook = lambda: state[0]
        sys.modules["antenv.axon_hooks"] = hooks
        hooks.set_axon_ntff_profile_hook(
            tb._ntff_profile_via_ctypes("/opt/axon/libaxon_pjrt.so"))
    return _run(inputs, trace=True, trace_cores=trace_cores)


# revision 9
# speedup vs baseline: 1.1132x; 1.0150x over previous
"""Bucket (block-diagonal) attention layer for Trainium2, 8 NeuronCores SPMD.

Sharding: data-parallel over batch (4) x tensor-parallel over head groups (2).
Core c = b*2 + g handles batch b, global heads [g*8, g*8+8).

Per-core math (local out dim 512 = 8 heads x 64):
  qT[dl, t] = sum_k Wq[g*512+dl, k] * x[b, t, k]  (+ bq)   [transposed layout]
  kT[dl, t] = likewise (bk dropped: constant-per-row score shifts cancel in
              softmax -- only bq enters scores via bq . k_j)
  v[t, dl]  = fp8-e4m3 DoubleRow matmul (x*SX, Wv*SW quantized on host); the
              ones-columns are memset to ALPHA=SX*SW so attended/den descales
              exactly for free.
  scoresT[kt, qt] = matmul(lhsT=kT_head, rhs=qT_head)      (K=64)
  expT = exp(scoresT) in bf16 (no max subtraction; logits sigma ~3.3)
  att[qt, 0:64], den[qt] = matmul(lhsT=expT, rhs=[v_head | alpha])  (bf16)
  y = att / den + (x_slice + bv)   [residual + bv folded on host, fp16]

Perf structure vs v7 baseline (219us):
 - V projection runs fp8 DoubleRow: 4 K=256 matmuls per 128-token tile
   instead of 8 K=128 fp16 matmuls (PE ~1.5x on that third of proj work)
 - score matmuls emitted h=0..7 so consecutive MMs target alternating
   row-groups (K=64 -> rows 0-63 / 64-127) and stream concurrently
 - xt DRAM layout made tt-contiguous; head DMAs reordered/split so the PE
   never waits >1 load at the start
 - y written fp16 (halves writeback bytes; host casts to fp32)
"""

import json
import sys

import numpy as np

FP16 = np.float16

B, S, D = 4, 4096, 1024
H, NB = 16, 32
HG = 2            # head groups (tensor parallel over heads)
NCORES = B * HG   # 8
DL = D // HG      # 512 local output dims per core
HL = H // HG      # 8 local heads
HD = D // H       # 64 head dim
BS = S // NB      # 128 bucket size
KC = D // 128     # 8 contraction chunks of 128
KH = KC // 2      # 4 chunks per a/b half tile
CC = D // 256     # 4 fp8 DoubleRow contraction chunks of 256
NQ = 4            # token quarters processed as pipeline phases
TOKQ = S // NQ    # 1024 tokens per quarter
NBQ = TOKQ // BS  # 8 buckets per quarter
OD = DL // 128    # 4 out-dim partition tiles for qT/kT
VW = 66           # per-head block width in v tiles: 64 data + 1 ones + 1 pad

SX = 28.0         # fp8 scale for x  (|x|max*28 ~ 148 < 240)
SW = 768.0        # fp8 scale for Wv (|Wv|max*768 ~ 81 < 240)
ALPHA = SX * SW   # 21504 = 1.3125*2^14, exact in bf16
V_SWI = True      # DoubleRowSwInterleave (contiguous LDWEIGHTS) vs DoubleRow

_built = None     # cached (nc,) so repeated kernel() calls reuse the program


def _apply_waitfix():
    """This container's walrus accepts at most ONE sem wait per instruction.
    Post-process the BIR json: hoist extra waits onto injected wait-only
    EventSemaphore instructions just before the owning instruction."""
    import concourse.bass as bass

    if getattr(bass.Bass, "_waitfix_applied", False):
        return
    orig = bass.Bass.to_json_bytes

    def _split(m):
        n = 0
        for f in m["functions"]:
            for blk in f["blocks"]:
                out = []
                for inst in blk["instructions"]:
                    si = inst.get("sync_info")
                    if si and si.get("on_wait") and len(si["on_wait"]) > 1:
                        waits = si["on_wait"]
                        si["on_wait"] = waits[-1:]
                        for k, w in enumerate(waits[:-1]):
                            out.append({
                                "debug": inst.get("debug", 0),
                                "engine": inst["engine"],
                                "ins": [],
                                "outs": [],
                                "name": f"wfix{n}_{k}_{inst['name']}",
                                "opcode": "EventSemaphore",
                                "sync_info": {"on_update": [], "on_wait": [w]},
                            })
                        n += 1
                    out.append(inst)
                blk["instructions"] = out
        return n

    def patched(self):
        m = json.loads(orig(self))
        _split(m)
        return json.dumps(m).encode()

    bass.Bass.to_json_bytes = patched
    bass.Bass._waitfix_applied = True


def _build():
    global _built
    if _built is not None:
        return _built

    _apply_waitfix()
    import concourse.bass as bass
    import concourse.tile as tile
    from concourse import mybir
    from concourse.bass import ts

    f32 = mybir.dt.float32
    fp16 = mybir.dt.float16
    bf16 = mybir.dt.bfloat16
    f8e4 = mybir.dt.float8e4
    Act = mybir.ActivationFunctionType
    Alu = mybir.AluOpType
    DR = (mybir.MatmulPerfMode.DoubleRowSwInterleave if V_SWI
          else mybir.MatmulPerfMode.DoubleRow)

    # All inputs are host-side pre-arranged partition-major so every DMA
    # reads large contiguous spans per partition.
    #   xt  [p, q, tt, ab, kk, t]: x.T[(ab*KH+kk)*128+p, q*1024+tt*512+t] fp16
    #   xv8 [p, q, cc, i, t]:      e4m3(SX * x.T[cc*256+i*128+p, q*1024+t])
    #   wq/wk [p, kk, d]:          W.T[kk*128+p, d] fp16
    #   wv8 [p, cc, i, d]:         e4m3(SW * Wv.T[cc*256+i*128+p, d])
    #   xres[p, nb, d]:            x[nb*128+p, d] + bv[d] fp16
    nc = bass.Bass()
    xt = nc.dram_tensor("xt", [128, NQ * 2 * 2 * KH * 512], fp16,
                        kind="ExternalInput")
    xv8 = nc.dram_tensor("xv8", [128, NQ * CC * 2 * TOKQ], f8e4,
                         kind="ExternalInput")
    wq = nc.dram_tensor("wq", [128, KC * DL], fp16, kind="ExternalInput")
    wk = nc.dram_tensor("wk", [128, KC * DL], fp16, kind="ExternalInput")
    wv8d = nc.dram_tensor("wv8", [128, CC * 2 * DL], f8e4,
                          kind="ExternalInput")
    bqt = nc.dram_tensor("bq", [128, OD], f32, kind="ExternalInput")
    xres = nc.dram_tensor("xres", [128, NB * DL], fp16, kind="ExternalInput")
    y = nc.dram_tensor("y", [S, DL], fp16, kind="ExternalOutput")

    with tile.TileContext(nc) as tc:
        with (
            tc.tile_pool(name="wpool", bufs=1) as wpool,
            tc.tile_pool(name="xtp", bufs=2) as xtp,
            tc.tile_pool(name="xvp", bufs=2) as xvp,
            tc.tile_pool(name="qtp", bufs=2 * OD) as qtp,
            tc.tile_pool(name="ktp", bufs=2 * OD) as ktp,
            tc.tile_pool(name="vp", bufs=2 * NBQ) as vpool,
            tc.tile_pool(name="ep", bufs=4) as epool,
            tc.tile_pool(name="yp", bufs=3) as ypool,
            tc.tile_pool(name="xrp", bufs=2) as xrpool,
            tc.tile_pool(name="rp", bufs=8) as rpool,
            tc.tile_pool(name="ps_p", bufs=2, space="PSUM") as ps_p,
            tc.tile_pool(name="ps_s", bufs=4, space="PSUM") as ps_s,
            tc.tile_pool(name="ps_a", bufs=2, space="PSUM") as ps_a,
        ):
            # --- PE warm-up: dummy matmuls on zeros during the DMA head so
            # the HAM clock-gate releases (1.2->2.4GHz) before real work ---
            warm = wpool.tile([128, 640], fp16, tag="warm")
            nc.vector.memset(warm[:], 0.0)
            pwarm = ps_s.tile([128, 512], f32, tag="ps", name="pwarm")
            for i in range(8):
                nc.tensor.matmul(pwarm[:], warm[:, 512:640], warm[:, 0:512],
                                 start=(i == 0), stop=(i == 7))
            wsink = wpool.tile([128, 1], f32, tag="wsink")
            nc.vector.reciprocal(wsink[:], pwarm[:, 0:1])

            xt6 = xt[:, :].rearrange(
                "p (q tt ab kk t) -> p q tt ab kk t", q=NQ, tt=2, ab=2, kk=KH)
            if V_SWI:
                # (p, q, cc, vt, j, b): A/B pair-interleaved, tokens reversed
                xv5 = xv8[:, :].rearrange(
                    "p (q cc vt j b) -> p q cc vt j b",
                    q=NQ, cc=CC, vt=NBQ, j=128)
            else:
                xv5 = xv8[:, :].rearrange(
                    "p (q cc i t) -> p q cc i t", q=NQ, cc=CC, i=2)
            xr3 = xres[:, :].rearrange("p (nb d) -> p nb d", d=DL)
            wqap = wq[:, :].rearrange("p (kk d) -> p kk d", kk=KC)
            wkap = wk[:, :].rearrange("p (kk d) -> p kk d", kk=KC)
            wv3 = wv8d[:, :].rearrange("p (cc i d) -> p cc i d", cc=CC, i=2)

            state = {}  # per-quarter tiles: xt, xv, xr, qt, kt, v, ex

            def load_xt(q):
                tiles = []
                for tt in range(2):
                    for ab in range(2):
                        t = xtp.tile([128, KH, 512], fp16,
                                     tag=f"xt{tt}{ab}", name=f"xt{tt}{ab}")
                        nc.sync.dma_start(out=t[:], in_=xt6[:, q, tt, ab, :, :])
                        tiles.append(t)
                state.setdefault(q, {})["xt"] = tiles

            def load_xv(q):
                if V_SWI:
                    t = xvp.tile([128, CC, NBQ, 128, 2], f8e4,
                                 tag="xv", name="xv")
                    nc.sync.dma_start(out=t[:], in_=xv5[:, q, :, :, :, :])
                else:
                    t = xvp.tile([128, CC, 2, TOKQ], f8e4, tag="xv", name="xv")
                    nc.sync.dma_start(out=t[:], in_=xv5[:, q, :, :, :])
                state[q]["xv"] = t

            def load_xr(q):
                t = xrpool.tile([128, NBQ, DL], fp16, tag="xres", name="xres")
                nc.sync.dma_start(
                    out=t[:], in_=xr3[:, q * NBQ:(q + 1) * NBQ, :])
                state[q]["xr"] = t

            # --- head: interleave weight/x loads so the first q-units can
            # start after ~1MB and never stall afterwards ---
            bq_sb = wpool.tile([128, OD], f32, tag="bq")
            nc.sync.dma_start(out=bq_sb[:], in_=bqt[:, :])
            wq_a = wpool.tile([128, KH, DL], fp16, tag="wqa", name="wqa")
            wq_b = wpool.tile([128, KH, DL], fp16, tag="wqb", name="wqb")
            wk_a = wpool.tile([128, KH, DL], fp16, tag="wka", name="wka")
            wk_b = wpool.tile([128, KH, DL], fp16, tag="wkb", name="wkb")
            wv8_t = wpool.tile([128, CC, 2, DL], f8e4, tag="wv8", name="wv8")
            xt00a = xtp.tile([128, KH, 512], fp16, tag="xt00", name="xt00")
            xt00b = xtp.tile([128, KH, 512], fp16, tag="xt01", name="xt01")
            xt01a = xtp.tile([128, KH, 512], fp16, tag="xt10", name="xt10")
            xt01b = xtp.tile([128, KH, 512], fp16, tag="xt11", name="xt11")
            state[0] = {"xt": [xt00a, xt00b, xt01a, xt01b]}
            nc.sync.dma_start(out=wq_a[:], in_=wqap[:, 0:KH, :])
            nc.sync.dma_start(out=xt00a[:], in_=xt6[:, 0, 0, 0, :, :])
            nc.sync.dma_start(out=wq_b[:], in_=wqap[:, KH:KC, :])
            nc.sync.dma_start(out=xt00b[:], in_=xt6[:, 0, 0, 1, :, :])
            nc.sync.dma_start(out=wk_a[:], in_=wkap[:, 0:KH, :])
            nc.sync.dma_start(out=wk_b[:], in_=wkap[:, KH:KC, :])
            nc.sync.dma_start(out=xt01a[:], in_=xt6[:, 0, 1, 0, :, :])
            nc.sync.dma_start(out=xt01b[:], in_=xt6[:, 0, 1, 1, :, :])
            nc.sync.dma_start(out=wv8_t[:], in_=wv3[:, :, :, :])
            load_xv(0)
            load_xr(0)

            def proj_units(q):
                """Yield 24 emission units: 16 q/k groups + 8 v groups."""
                st = state[q]
                xts = st["xt"]
                qt_sb = [qtp.tile([128, TOKQ], fp16, tag="qt", name="qt")
                         for _ in range(OD)]
                kt_sb = [ktp.tile([128, TOKQ], fp16, tag="kt", name="kt")
                         for _ in range(OD)]
                v_sb = [vpool.tile([128, HL * VW], bf16, tag="v", name="v")
                        for _ in range(NBQ)]
                st["qt"], st["kt"], st["v"] = qt_sb, kt_sb, v_sb

                def qk_unit(which, od, tt):
                    def emit():
                        dst = qt_sb if which == "q" else kt_sb
                        wa, wb = (wq_a, wq_b) if which == "q" else (wk_a, wk_b)
                        p = ps_p.tile([128, 512], f32, tag="pp", name="pp")
                        for kk in range(KC):
                            w_ap = (wa if kk < KH else wb)[
                                :, kk % KH, ts(od, 128)]
                            nc.tensor.matmul(
                                p[:], w_ap, xts[tt * 2 + kk // KH][:, kk % KH, :],
                                start=(kk == 0), stop=(kk == KC - 1))
                        if which == "q":
                            nc.scalar.activation(
                                dst[od][:, ts(tt, 512)], p[:], Act.Identity,
                                bias=bq_sb[:, od:od + 1], scale=1.0)
                        else:
                            nc.scalar.copy(dst[od][:, ts(tt, 512)], p[:])
                    return emit

                def v_unit(vt):
                    def emit():
                        xv = st["xv"]
                        p = ps_p.tile([128, 512], f32, tag="pp", name="pp")
                        for cc in range(CC):
                            lhsT = (xv[:, cc, vt, :, :] if V_SWI
                                    else xv[:, cc, :, ts(vt, 128)])
                            nc.tensor.matmul(
                                p[:], lhsT, wv8_t[:, cc, :, :],
                                start=(cc == 0), stop=(cc == CC - 1),
                                perf_mode=DR)
                        vt_sb = v_sb[vt]
                        v3 = vt_sb[:].rearrange("p (h c) -> p h c", c=VW)
                        nc.vector.memset(v3[:, :, 64:66], ALPHA)
                        nc.vector.tensor_copy(
                            v3[:, :, 0:64],
                            p[:].rearrange("p (h c) -> p h c", c=HD))
                    return emit

                # tt-major so quarter 0 can start on the first xt tile;
                # q before k so the wk DMA hides under the q-unit stream
                units = []
                for tt in range(2):
                    for od in range(OD):
                        units.append(qk_unit("q", od, tt))
                    for od in range(OD):
                        units.append(qk_unit("k", od, tt))
                for vt in range(NBQ):
                    units.append(v_unit(vt))
                return units

            def attn_scores(q, bk):
                """Part 1: scores matmuls + batched EXP for one bucket."""
                st = state[q]
                qt_sb, kt_sb = st["qt"], st["kt"]
                col = ts(bk, BS)  # token slice within quarter
                se = ps_s.tile([128, 512], f32, tag="ps", name="ps_e")
                so = ps_s.tile([128, 512], f32, tag="ps", name="ps_o")
                # h order 0..7: consecutive MMs target alternating row-groups
                # (rows 0-63 / 64-127) so pairs stream concurrently on the PE
                for h in range(HL):
                    od, po = h // 2, (h % 2) * 64
                    bank = se if h % 2 == 0 else so
                    nc.tensor.matmul(
                        bank[:, ts(h // 2, 128)],
                        kt_sb[od][po:po + 64, col],
                        qt_sb[od][po:po + 64, col],
                        start=True, stop=True)
                ex_e = epool.tile([128, 512], bf16, tag="ex", name="ex_e")
                ex_o = epool.tile([128, 512], bf16, tag="ex", name="ex_o")
                nc.scalar.activation(ex_e[:], se[:], Act.Exp)
                nc.scalar.activation(ex_o[:], so[:], Act.Exp)
                st.setdefault("ex", {})[bk] = (ex_e, ex_o)

            def attn_out(q, bk):
                """Part 2: attended matmuls + normalize + residual + out."""
                st = state[q]
                v_sb = st["v"]
                ex_e, ex_o = st["ex"].pop(bk)
                tok0 = q * TOKQ
                xr = st["xr"][:, bk, :]
                pe = ps_a.tile([128, HL // 2 * VW], f32, tag="pa", name="pa_e")
                po_ = ps_a.tile([128, HL // 2 * VW], f32, tag="pa", name="pa_o")
                for h in (0, 2, 4, 6, 1, 3, 5, 7):
                    ex = ex_e if h % 2 == 0 else ex_o
                    bank = pe if h % 2 == 0 else po_
                    slot = h // 2
                    nc.tensor.matmul(
                        bank[:, slot * VW:slot * VW + VW],
                        ex[:, ts(slot, 128)],
                        v_sb[bk][:, h * VW:(h + 1) * VW],
                        start=True, stop=True)
                yt = ypool.tile([128, DL], fp16, tag="yt")
                for par, bank in ((0, pe), (1, po_)):
                    pav = bank[:].rearrange("p (h c) -> p h c", c=VW)
                    rc = rpool.tile([128, HL // 2], f32, tag="rc")
                    nc.vector.reciprocal(
                        rc[:].unsqueeze(2), pav[:, :, 64:65])
                    ytv = yt[:].rearrange(
                        "p (h two c) -> p h two c", two=2, c=HD)[:, :, par, :]
                    rcb = rc[:].unsqueeze(2).broadcast_to((128, HL // 2, HD))
                    nc.vector.tensor_tensor(
                        out=ytv, in0=pav[:, :, 0:HD], in1=rcb, op=Alu.mult)
                # residual add on the (idle) gpsimd engine -- DVE is the
                # bucket-phase bottleneck and this op only feeds the y DMA
                nc.gpsimd.tensor_tensor(
                    out=yt[:], in0=yt[:], in1=xr[:], op=Alu.add)
                nc.sync.dma_start(
                    out=y[tok0 + bk * BS:tok0 + (bk + 1) * BS, :], in_=yt[:])

            # --- emission: per quarter, 16 q/k units then for each bucket
            # [v-unit, scores, attended(bk-1)] -- the EXP latency of bucket
            # bk hides under the v projection of bucket bk+1.  The last
            # bucket's attended spills into the next quarter's first unit.
            pending = None
            for q in range(NQ):
                units = proj_units(q)
                for i in range(2 * OD * 2):
                    units[i]()
                    if i == 0 and pending is not None:
                        attn_out(*pending)
                        pending = None
                    if i == 7 and q + 1 < NQ:
                        load_xt(q + 1)
                        load_xv(q + 1)
                        load_xr(q + 1)
                for bk in range(NBQ):
                    units[16 + bk]()
                    attn_scores(q, bk)
                    if bk > 0:
                        attn_out(q, bk - 1)
                pending = (q, NBQ - 1)
            attn_out(*pending)

    _built = nc
    return nc


def _prep_in_maps(x, Wq, bq, Wk, bk, Wv, bv):
    import ml_dtypes

    E4M3 = ml_dtypes.float8_e4m3

    x = np.asarray(x, np.float32)
    Wq = np.asarray(Wq, np.float32)
    Wv = np.asarray(Wv, np.float32)
    Wk = np.asarray(Wk, np.float32)
    bq = np.asarray(bq, np.float32)
    bv = np.asarray(bv, np.float32)

    def _xt_layout(a):  # a: [D, S] fp16 -> (p, q, tt, ab, kk, t)
        return np.ascontiguousarray(
            a.reshape(2, KH, 128, NQ, 2, 512).transpose(2, 3, 4, 0, 1, 5)
        ).reshape(128, NQ * 2 * 2 * KH * 512)

    def _xv8_layout(aT):  # aT: [D, S] fp32 -> e4m3*SX, DoubleRow pair layout
        a = np.asarray(aT * np.float32(SX), E4M3)
        if V_SWI:
            # (p, q, cc, vt, j, b) = a[cc*256 + b*128 + p,
            #                          q*1024 + vt*128 + (127 - j)]
            v = a.reshape(CC, 2, 128, NQ, NBQ, 128)[..., ::-1]
            return np.ascontiguousarray(
                v.transpose(2, 3, 0, 4, 5, 1)
            ).reshape(128, NQ * CC * 2 * TOKQ)
        return np.ascontiguousarray(
            a.reshape(CC, 2, 128, NQ, TOKQ).transpose(2, 3, 0, 1, 4)
        ).reshape(128, NQ * CC * 2 * TOKQ)

    def _w_layout(a):  # a: [D, DL] fp16 -> (p, kk, d)
        return np.ascontiguousarray(
            a.reshape(KC, 128, DL).transpose(1, 0, 2)).reshape(128, KC * DL)

    def _wv8_layout(aT):  # aT: [D, DL] fp32 -> (p, cc, i, d) e4m3*SW
        a = np.asarray(aT * np.float32(SW), E4M3)
        return np.ascontiguousarray(
            a.reshape(CC, 2, 128, DL).transpose(2, 0, 1, 3)
        ).reshape(128, CC * 2 * DL)

    xT_b = [np.ascontiguousarray(x[b].T) for b in range(B)]
    xt_b = [_xt_layout(t.astype(FP16)) for t in xT_b]
    xv8_b = [_xv8_layout(t) for t in xT_b]
    wq_g, wk_g, wv8_g, bq_g = [], [], [], []
    for g in range(HG):
        sl = slice(g * DL, (g + 1) * DL)
        wq_g.append(_w_layout(Wq[sl, :].T.astype(FP16)))
        wk_g.append(_w_layout(Wk[sl, :].T.astype(FP16)))
        wv8_g.append(_wv8_layout(np.ascontiguousarray(Wv[sl, :].T)))
        bq_g.append(np.ascontiguousarray(
            bq[sl].reshape(DL // 128, 128).T).astype(np.float32))

    in_maps = []
    for c in range(NCORES):
        b, g = c // HG, c % HG
        sl = slice(g * DL, (g + 1) * DL)
        xres = (x[b][:, sl] + bv[None, sl]).astype(FP16)  # [S, DL]
        xres = np.ascontiguousarray(
            xres.reshape(NB, 128, DL).transpose(1, 0, 2)
        ).reshape(128, NB * DL)
        in_maps.append({
            "xt": xt_b[b], "xv8": xv8_b[b], "wq": wq_g[g], "wk": wk_g[g],
            "wv8": wv8_g[g], "bq": bq_g[g], "xres": xres,
        })
    return in_maps


def _gather(results):
    out = np.empty((B, S, D), np.float32)
    for c, r in enumerate(results):
        b, g = c // HG, c % HG
        out[b, :, g * DL:(g + 1) * DL] = np.asarray(r["y"], np.float32)
    return out


def _run(inputs, trace=False, trace_cores=None):
    nc = _build()
    from concourse.bass_utils import run_bass_kernel_spmd

    in_maps = _prep_in_maps(**inputs)
    res = run_bass_kernel_spmd(
        nc, in_maps, core_ids=list(range(NCORES)), trace=trace,
        trace_cores=trace_cores)
    return _gather(res.results), res


def kernel(**inputs):
    out, _ = _run(inputs, trace=False)
    return out


def kernel_traced(trace_cores=None, **inputs):
    """For test.py: returns (output, BassKernelResults with exec_time_ns)."""
    import types
    import trn_agent_boot.trn_boot as tb

    if "antenv.axon_hooks" not in sys.modules:
        hooks = types.ModuleType("antenv.axon_hooks")
        state = [None]
        hooks.set_axon_ntff_profile_hook = lambda h: state.__setitem__(0, h)
        hooks.get_axon_ntff_profile_hook = lambda: state[0]
        sys.modules["antenv.axon_hooks"] = hooks
        hooks.set_axon_ntff_profile_hook(
            tb._ntff_profile_via_ctypes("/opt/axon/libaxon_pjrt.so"))
    return _run(inputs, trace=True, trace_cores=trace_cores)
